# revision 2
# baseline (speedup 1.0000x reference)
"""ARD-RBF covariance kernel for Trainium2 (Bass/Tile), 8-core row-parallel.

Math (matches the reference):
    s  = exp(-weights[:, 0])                      # (D,) inverse lengthscales
    Us = U * s ; Vs = V * s
    sq[i, j] = ||Us_i||^2 + ||Vs_j||^2 - 2 Us_i . Vs_j
    K[i, j]  = exp(2*sn) * exp(-0.5 * max(sq, 0))

Device strategy (per core, rows sharded 8 ways):
    One augmented matmul computes sq directly in PSUM:
      lhsT (K=18 x 128) rows: [-2*s*U^T block ; ||Us||^2 row ; ones row]
      rhs  (K=18 x 512) rows: [ s*V^T        ; ones row     ; ||Vs||^2 row]
    Then a single ScalarE activation per tile computes
      out = Exp(-0.5 * psum + 2*sn)   (PSUM -> SBUF, fp16), and DMA stores.

Perf notes:
  - Output is written as fp16 (abs err ~5e-4 << 2e-2 tolerance) and widened
    to f32 on the host: halves HBM store traffic to 16 MB/core (~45 us at
    358 GB/s/core).
  - Store DMAs alternate between the qSP HWDGE ring (nc.sync) and the SWDGE
    ring (nc.gpsimd) so the ~2 us per-DMA completion-receipt stall of one
    ring overlaps the data phase of the other; a single ring serializes
    those stalls (that was the old bottleneck: 8.4 us per 1 MB store).
  - Steady state is then bound by ScalarE exp throughput: 8.39M elem/core
    at 1 elem/lane/cycle @ 1.2 GHz ~= 55-60 us.
"""

import numpy as np

import concourse.bacc as bacc
import concourse.bass as bass  # noqa: F401  (AP helpers)
import concourse.mybir as mybir
import concourse.tile as tile

N, M, D = 8192, 8192, 16
N_CORES = 8
ROWS = N // N_CORES  # 1024 rows of U per core
P = 128              # output partitions per row block
FREE = 512           # matmul moving free dim (one PSUM bank of f32)
QUAD = 2048          # ACT chunk: 4 banks
K = D + 2            # augmented contraction dim

F32 = mybir.dt.float32
F16 = mybir.dt.float16
AF = mybir.ActivationFunctionType


def build_program(rows=ROWS, m_cols=M, repeats=1):
    """Build the per-core Bass program. rows/m_cols shrinkable for sim."""
    rb = rows // P
    nq = m_cols // QUAD

    nc = bacc.Bacc()
    ut = nc.declare_dram_parameter("ut", [D, rows], F32, isOutput=False)
    vt = nc.declare_dram_parameter("vt", [D, m_cols], F32, isOutput=False)
    w = nc.declare_dram_parameter("w", [D, 1], F32, isOutput=False)
    sn = nc.declare_dram_parameter("sn", [1, 1], F32, isOutput=False)
    out = nc.declare_dram_parameter("out", [rows, m_cols], F16, isOutput=True)

    with tile.TileContext(nc) as tc:
        with (
            tc.tile_pool(name="singles", bufs=1) as singles,
            tc.tile_pool(name="scratch", bufs=2) as scratch,
            tc.tile_pool(name="psum_pool", bufs=2, space="PSUM") as psum_pool,
            tc.tile_pool(name="obuf_pool", bufs=3) as obuf_pool,
        ):
            # --- scale factors -------------------------------------------
            wt = singles.tile([D, 1], F32)
            nc.sync.dma_start(wt[:], w[:])
            s_t = singles.tile([D, 1], F32)
            nc.scalar.activation(s_t[:], wt[:], AF.Exp, scale=-1.0)  # s = exp(-w)
            s2_t = singles.tile([D, 1], F32)
            nc.scalar.mul(s2_t[:], s_t[:], -2.0)                     # -2s

            snb = singles.tile([P, 1], F32)
            nc.gpsimd.dma_start(snb[:], sn[:].to_broadcast((P, 1)))
            bias2 = singles.tile([P, 1], F32)
            nc.scalar.mul(bias2[:], snb[:], 2.0)                     # 2*sn

            ones16 = singles.tile([D, 1], F32)
            nc.vector.memset(ones16[:], 1.0)
            quart16 = singles.tile([D, 1], F32)
            nc.vector.memset(quart16[:], 0.25)

            # Compute-engine SBUF APs must start at partition 0/32/64/96, so
            # the augmented rows (16, 17) are built in partition-0 scratch
            # tiles and DMA'd into place (DMA has no partition restriction).
            onesrow = singles.tile([1, QUAD], F32)
            nc.vector.memset(onesrow[:], 1.0)

            # --- lhsT: L = [-2 s U^T ; u2 ; 1] ---------------------------
            # L/R carry 4 copies of the K=18 operand at partitions 0/32/64/96
            # so 4 matmuls can run concurrently in the PE's four 32-row
            # groups (tile_position row tiling) — hides the fp32 LDWEIGHTS
            # and 2-pass matmul cost behind concurrent streaming.
            L = singles.tile([3 * 32 + K, rows], F32)
            nc.sync.dma_start(L[0:D, :], ut[:])
            # tensor_tensor with a broadcast AP rather than tensor_scalar:
            # TensorScalarPtr only has one sync-wait slot in the ISA.
            nc.vector.tensor_mul(L[0:D, :], L[0:D, :], s2_t.to_broadcast((D, rows)))
            for c in range(rows // QUAD + (1 if rows % QUAD else 0)):
                w_ = min(QUAD, rows - c * QUAD)
                nc.sync.dma_start(
                    L[D + 1 : D + 2, c * QUAD : c * QUAD + w_], onesrow[:, :w_]
                )
            qU = singles.tile([D, rows], F32)
            nc.vector.tensor_mul(qU[:], L[0:D, :], L[0:D, :])        # 4 s^2 U^2
            u2row = singles.tile([1, rows], F32)
            for c in range(rows // FREE):
                ps = psum_pool.tile([P, QUAD], F32, tag="ps", name="ps")
                nc.tensor.matmul(
                    ps[0:1, 0:FREE], quart16[:], qU[:, c * FREE : (c + 1) * FREE],
                    start=True, stop=True,
                )
                nc.vector.tensor_copy(
                    u2row[:, c * FREE : (c + 1) * FREE], ps[0:1, 0:FREE]
                )
            nc.sync.dma_start(L[D : D + 1, :], u2row[:])
            for g in range(1, 4):
                nc.sync.dma_start(L[32 * g : 32 * g + K, :], L[0:K, :])

            # --- rhs: R = [s V^T ; 1 ; v2], built per 2048-col group -----
            R = singles.tile([3 * 32 + K, m_cols], F32)
            nc.sync.dma_start(R[0:D, :], vt[:])
            for g in range(m_cols // QUAD):
                gsl = slice(g * QUAD, (g + 1) * QUAD)
                nc.vector.tensor_mul(
                    R[0:D, gsl], R[0:D, gsl], s_t.to_broadcast((D, QUAD))
                )
                nc.sync.dma_start(R[D : D + 1, gsl], onesrow[:])
                qvg = scratch.tile([D, QUAD], F32, tag="qvg", name="qvg")
                nc.vector.tensor_mul(qvg[:], R[0:D, gsl], R[0:D, gsl])  # s^2 V^2
                vrow = scratch.tile([1, QUAD], F32, tag="vrow", name="vrow")
                for c in range(QUAD // FREE):
                    ps = psum_pool.tile([P, QUAD], F32, tag="ps", name="ps")
                    nc.tensor.matmul(
                        ps[0:1, 0:FREE], ones16[:], qvg[:, c * FREE : (c + 1) * FREE],
                        start=True, stop=True,
                    )
                    nc.vector.tensor_copy(
                        vrow[:, c * FREE : (c + 1) * FREE], ps[0:1, 0:FREE]
                    )
                nc.sync.dma_start(R[D + 1 : D + 2, gsl], vrow[:])
            for g in range(1, 4):
                nc.sync.dma_start(R[32 * g : 32 * g + K, :], R[0:K, :])

            # --- main loop ----------------------------------------------
            for _rep in range(repeats):
                for m in range(rb):
                    ob = obuf_pool.tile([P, m_cols], F16, tag="ob", name="ob")
                    for q in range(nq):
                        ps = psum_pool.tile([P, QUAD], F32, tag="ps", name="ps")
                        for k in range(QUAD // FREE):
                            n = q * (QUAD // FREE) + k
                            nc.tensor.matmul(
                                ps[:, k * FREE : (k + 1) * FREE],
                                L[32 * k : 32 * k + K, m * P : (m + 1) * P],
                                R[32 * k : 32 * k + K, n * FREE : (n + 1) * FREE],
                                start=True, stop=True,
                                tile_position=(32 * k, 0),
                            )
                        nc.scalar.activation(
                            ob[:, q * QUAD : (q + 1) * QUAD], ps[:],
                            AF.Exp, bias=bias2[:], scale=-0.5,
                        )
                        # store each quad as soon as its ACT lands; alternate
                        # between the qSP HWDGE ring and the SWDGE (gpsimd)
                        # ring so per-DMA completion stalls overlap across
                        # rings instead of serializing on one.
                        eng = nc.sync if (m * nq + q) % 2 == 0 else nc.gpsimd
                        eng.dma_start(
                            out[m * P : (m + 1) * P, q * QUAD : (q + 1) * QUAD],
                            ob[:, q * QUAD : (q + 1) * QUAD],
                        )

    nc.compile()  # bacc lowering: splits multi-waits, reg alloc, etc.
    return nc


_PROGRAM_CACHE = {}


def get_program(rows=ROWS, m_cols=M, repeats=1):
    key = (rows, m_cols, repeats)
    if key not in _PROGRAM_CACHE:
        _PROGRAM_CACHE[key] = build_program(rows, m_cols, repeats)
    return _PROGRAM_CACHE[key]


def make_in_maps(U, V, weights, sn):
    U = np.ascontiguousarray(np.asarray(U, dtype=np.float32))
    V = np.ascontiguousarray(np.asarray(V, dtype=np.float32))
    w = np.ascontiguousarray(np.asarray(weights, dtype=np.float32).reshape(D, 1))
    snr = np.asarray(sn, dtype=np.float32).reshape(1, 1)
    vt = np.ascontiguousarray(V.T)
    in_maps = []
    for c in range(N_CORES):
        ut = np.ascontiguousarray(U[c * ROWS : (c + 1) * ROWS].T)
        in_maps.append({"ut": ut, "vt": vt, "w": w, "sn": snr})
    return in_maps


def kernel(U, V, weights, sn):
    from concourse.bass_utils import run_bass_kernel_spmd

    nc = get_program()
    in_maps = make_in_maps(U, V, weights, sn)
    res = run_bass_kernel_spmd(nc, in_maps, core_ids=list(range(N_CORES)))
    return np.concatenate(
        [np.asarray(r["out"]).astype(np.float32) for r in res.results], axis=0
    )
